# revision 6
# baseline (speedup 1.0000x reference)
"""AUSTGN (time/dist-gated LSTM variant) Trainium2 Bass kernel.

Problem (hardcoded): B=256, T=512, IPT=512, HID=512, Q=64.
Sharding: data-parallel over batch across 8 cores (32 rows/core);
weights replicated. Each core runs the full T-step scan on its rows.

Per-core pipeline:
  - x projections (8 gate blocks) precomputed chunk-by-chunk with PE
    matmuls (x transposed on-chip via PE transpose); rank-1 Tt/Dt
    sigmoid terms via ACT per-partition scale; gate biases folded into
    the PSUM drains; td-gate additive terms injected via identity-MMs.
  - recurrence: per step, h^T (bf16) is the stationary operand; the 4
    gate matmuls go to one PSUM bank as 32-partition strips via
    tile_position col-groups (c-gate first so its tanh overlaps);
    pre-activations injected with a stacked-identity matmul; tanh and
    the final h product computed in TRANSPOSED layout so h^T feeds the
    next step directly (no h transpose on the critical path).
"""

import sys

sys.path.insert(0, "/opt/trn_rl_repo")

import numpy as np
import ml_dtypes

import concourse.bass as bass
import concourse.mybir as mybir
import concourse.tile as tile
from concourse import bass_utils, bacc

BF16 = mybir.dt.bfloat16
F32 = mybir.dt.float32
AF = mybir.ActivationFunctionType
ALU = mybir.AluOpType

B, T, IPT, HID = 256, 512, 512, 512
NCORES = 8
BS = B // NCORES          # 32 rows per core
TC = 16                   # timesteps per chunk
NCHUNK = T // TC          # 32
RT = TC // 4              # row-tiles (128 = 4 steps x 32 batch) per chunk

# gate order for fused weights / psum strips: 0=i, 1=f, 2=o, 3=c
# x-projection block order: i, f, o, c, t1, t2, d1, d2


def build_bass(reps=1):
    nc = bacc.Bacc("TRN2", target_bir_lowering=False, debug=False)

    x_d = nc.dram_tensor("x", [BS, T, 2 + IPT], F32, kind="ExternalInput")
    wall_d = nc.dram_tensor("wall", [IPT, 8 * HID], BF16, kind="ExternalInput")
    whf_d = nc.dram_tensor("whf", [HID, 4 * HID], BF16, kind="ExternalInput")
    # wtb: [Wt1, Wt2, Wd1, Wd2, Wto, Wdo] broadcast to 128 partitions (bf16)
    wtb_d = nc.dram_tensor("wtb", [6, 128, HID], BF16, kind="ExternalInput")
    # biasb: [bi, bf, bo, bc, bt1, bt2, bd1, bd2] broadcast (bf16)
    biasb_d = nc.dram_tensor("biasb", [8, 128, HID], BF16, kind="ExternalInput")
    ist_d = nc.dram_tensor("ist", [128, 32], BF16, kind="ExternalInput")
    ident_d = nc.dram_tensor("ident", [128, 128], BF16, kind="ExternalInput")

    hseq_d = nc.dram_tensor("hseq", [BS, T, HID], F32, kind="ExternalOutput")
    cout_d = nc.dram_tensor("c_out", [BS, HID], F32, kind="ExternalOutput")

    with tile.TileContext(nc) as tc:
        with (
            tc.tile_pool(name="const", bufs=1) as constp,
            tc.tile_pool(name="xin", bufs=3) as xinp,
            tc.tile_pool(name="prec", bufs=2) as precp,
            tc.tile_pool(name="work", bufs=2) as workp,
            tc.tile_pool(name="outp", bufs=2) as outp,
            tc.tile_pool(name="state", bufs=1) as statep,
            tc.tile_pool(name="psA", bufs=1, space=bass.MemorySpace.PSUM) as psAp,
            tc.tile_pool(name="psG", bufs=1, space=bass.MemorySpace.PSUM) as psGp,
            tc.tile_pool(name="psX", bufs=1, space=bass.MemorySpace.PSUM) as psXp,
            tc.tile_pool(name="psH", bufs=1, space=bass.MemorySpace.PSUM) as psHp,
        ):
            # ---------------- constants ----------------
            wall = constp.tile([128, 4 * 8 * HID], BF16, tag="wall")
            for k in range(4):
                nc.sync.dma_start(
                    wall[:, k * 8 * HID:(k + 1) * 8 * HID],
                    wall_d.ap()[128 * k:128 * (k + 1), :],
                )
            whf = constp.tile([128, 4 * 4 * HID], BF16, tag="whf")
            for k in range(4):
                nc.sync.dma_start(
                    whf[:, k * 4 * HID:(k + 1) * 4 * HID],
                    whf_d.ap()[128 * k:128 * (k + 1), :],
                )
            wtb = constp.tile([128, 6 * HID], BF16, tag="wtb")
            nc.sync.dma_start(wtb[:], wtb_d.ap().rearrange("g p c -> p g c"))
            biasb = constp.tile([128, 8 * HID], BF16, tag="biasb")
            nc.sync.dma_start(biasb[:], biasb_d.ap().rearrange("g p c -> p g c"))
            ist = constp.tile([128, 32], BF16, tag="ist")
            nc.sync.dma_start(ist[:], ist_d.ap())
            ident = constp.tile([128, 128], BF16, tag="ident")
            nc.sync.dma_start(ident[:], ident_d.ap())

            def wall_k(k, blk):  # K-tile k, x-projection block blk
                return wall[:, k * 8 * HID + blk * HID:k * 8 * HID + (blk + 1) * HID]

            def whf_k(k, g):
                return whf[:, k * 4 * HID + g * HID:k * 4 * HID + (g + 1) * HID]

            def wtb_g(g):
                return wtb[:, g * HID:(g + 1) * HID]

            def biasb_g(g):
                return biasb[:, g * HID:(g + 1) * HID]

            # ---------------- state ----------------
            hT = [
                statep.tile([128, 128], BF16, tag="hTa", name="hTa"),
                statep.tile([128, 128], BF16, tag="hTb", name="hTb"),
            ]
            # JC strips: [0:32]=j (tanh gate, per step), [32:64]=c state
            JC = [
                statep.tile([64, HID], BF16, tag="jca", name="jca"),
                statep.tile([64, HID], BF16, tag="jcb", name="jcb"),
            ]

            # chunk-local tensors, double buffered via pool bufs=2
            def alloc_chunk_tiles():
                pre = [precp.tile([128, 4 * HID], BF16, tag=f"pre{r}", name=f"pre{r}")
                       for r in range(RT)]
                td = [precp.tile([128, 2 * HID], BF16, tag=f"td{r}", name=f"td{r}")
                      for r in range(RT)]
                return pre, td

            # ---------------- precompute one row-tile ----------------
            def precompute_rowtile(chunk, r, pre, td):
                t0 = chunk * TC + 4 * r
                xf = xinp.tile([128, 2 + IPT], F32, tag="xf")
                nc.sync.dma_start(
                    xf[:], x_d.ap()[:, t0:t0 + 4, :].rearrange("b t c -> t b c")
                )
                xb = xinp.tile([128, 2 + IPT], BF16, tag="xb")
                nc.gpsimd.tensor_copy(xb[:], xf[:])
                tt_col = xf[:, 0:1]
                dt_col = xf[:, 1:2]

                # transpose x (128 rows, 512 cols) -> xT (4 K-blocks of (128,128))
                psx = psXp.tile([128, 512], BF16, tag="psx")
                for k in range(4):
                    nc.tensor.transpose(
                        psx[:, 128 * k:128 * (k + 1)],
                        xb[:, 2 + 128 * k:2 + 128 * (k + 1)],
                        ident[:],
                    )
                xT = xinp.tile([128, 512], BF16, tag="xT")
                nc.vector.tensor_copy(xT[:], psx[:])

                # o-gate rank-1 terms + bias: q3b = Tt*Wto + Dt*Wdo + bo
                q1 = workp.tile([128, HID], BF16, tag="q1")
                nc.vector.tensor_scalar(q1[:], wtb_g(4), tt_col, None, ALU.mult)
                q3b = workp.tile([128, HID], BF16, tag="q3b")
                nc.vector.scalar_tensor_tensor(
                    q3b[:], wtb_g(5), dt_col, q1[:], ALU.mult, ALU.add
                )
                nc.vector.tensor_add(q3b[:], q3b[:], biasb_g(2))

                # sigmoid outer terms for td gates: sb_g = sig(scale*W) + bias
                sb = []
                for bi, blk in enumerate([4, 5, 6, 7]):  # t1, t2, d1, d2
                    s = workp.tile([128, HID], BF16, tag=f"souter{bi}",
                                   name=f"souter{bi}")
                    nc.scalar.activation(
                        s[:], wtb_g(bi), AF.Sigmoid,
                        scale=tt_col if bi < 2 else dt_col,
                    )
                    nc.vector.tensor_add(s[:], s[:], biasb_g(blk))
                    sb.append(s)

                # pass A: gate blocks i,f,o,c (bias/q3b folded into drains)
                psa = [psAp.tile([128, HID], F32, tag=f"psa{g}", name=f"psa{g}")
                       for g in range(4)]
                for g in range(4):
                    for k in range(4):
                        nc.tensor.matmul(
                            psa[g][:],
                            xT[:, 128 * k:128 * (k + 1)],
                            wall_k(k, g),
                            start=(k == 0),
                            stop=(k == 3),
                        )
                for g in range(4):
                    dst = pre[r][:, g * HID:(g + 1) * HID]
                    addend = q3b[:] if g == 2 else biasb_g(g)
                    nc.vector.tensor_add(dst, psa[g][:], addend)

                # pass B: td blocks t1,t2,d1,d2 (sb injected via ident-matmul)
                psb = [psAp.tile([128, HID], F32, tag=f"psa{g}", name=f"psb{g}")
                       for g in range(4)]
                gtd = []
                for bi, blk in enumerate([4, 5, 6, 7]):
                    for k in range(4):
                        nc.tensor.matmul(
                            psb[bi][:],
                            xT[:, 128 * k:128 * (k + 1)],
                            wall_k(k, blk),
                            start=(k == 0),
                            stop=False,
                        )
                    nc.tensor.matmul(
                        psb[bi][:], ident[:], sb[bi][:], start=False, stop=True
                    )
                    g_ = workp.tile([128, HID], BF16, tag=f"gtd{bi}",
                                    name=f"gtd{bi}")
                    nc.scalar.activation(g_[:], psb[bi][:], AF.Sigmoid)
                    gtd.append(g_)
                nc.vector.tensor_mul(td[r][:, 0:HID], gtd[0][:], gtd[2][:])
                nc.vector.tensor_mul(td[r][:, HID:2 * HID], gtd[1][:], gtd[3][:])

            # ---------------- one recurrence step ----------------
            def step(chunk, s, pre, td, O):
                sg = chunk * TC + s          # global step (within rep)
                tau = s % 4
                r = s // 4
                cur = hT[sg % 2]
                nxt = hT[(sg + 1) % 2]
                jc_cur = JC[sg % 2]
                jc_nxt = JC[(sg + 1) % 2]

                # stage this step's td pair at base partition 0 (TT operands
                # must share base partition); no deps on the step chain.
                tds = workp.tile([32, 2 * HID], BF16, tag="tds")
                nc.vector.tensor_copy(tds[:], td[r][32 * tau:32 * (tau + 1), :])

                psg = psGp.tile([128, HID], F32, tag="psg")
                # c gate (strip 3) first so tanh_j overlaps remaining matmuls
                for g in [3, 0, 1, 2]:
                    for k in range(4):
                        nc.tensor.matmul(
                            psg[32 * g:32 * (g + 1), :],
                            cur[:, 32 * k:32 * (k + 1)],
                            whf_k(k, g),
                            start=(k == 0),
                            stop=False,
                            tile_position=(0, 32 * g),
                        )
                    nc.tensor.matmul(
                        psg[32 * g:32 * (g + 1), :],
                        ist[32 * tau:32 * (tau + 1), :],
                        pre[r][32 * tau:32 * (tau + 1), g * HID:(g + 1) * HID],
                        start=False,
                        stop=True,
                        tile_position=(32 * tau, 32 * g),
                    )

                # j (tanh gate) -> JC strip 0
                nc.scalar.activation(jc_cur[0:32, :], psg[96:128, :], AF.Tanh)
                sig = workp.tile([96, HID], BF16, tag="sig")
                nc.scalar.activation(sig[:], psg[0:96, :], AF.Sigmoid)

                # transpose ot while the DVE chain runs (PE idle)
                psot = psHp.tile([128, 128], BF16, tag="psot")
                for k in range(4):
                    nc.tensor.transpose(
                        psot[:, 32 * k:32 * (k + 1)],
                        sig[64:96, 128 * k:128 * (k + 1)],
                        ist[64:96, :],
                    )
                otT = workp.tile([128, 128], BF16, tag="otT")
                nc.vector.tensor_copy(otT[:], psot[:])

                ua = workp.tile([64, HID], BF16, tag="ua")
                nc.vector.tensor_mul(ua[:], sig[0:64, :], jc_cur[0:64, :])
                vv = workp.tile([64, HID], BF16, tag="vv")
                nc.vector.tensor_mul(vv[32:64, :], ua[0:32, :], tds[:, 0:HID])
                ch = workp.tile([32, HID], BF16, tag="ch")
                nc.vector.tensor_add(ch[:], ua[32:64, :], vv[32:64, :])
                vv2 = workp.tile([64, HID], BF16, tag="vv2")
                nc.vector.tensor_mul(vv2[32:64, :], ua[0:32, :], tds[:, HID:2 * HID])
                nc.vector.tensor_add(jc_nxt[32:64, :], ua[32:64, :], vv2[32:64, :])

                # critical path: ch -> chT (PE) -> tanh (ACT, from PSUM) ->
                # hT_next = otT * tanh(chT)  (DVE)
                psc = psHp.tile([128, 128], BF16, tag="psc")
                for k in range(4):
                    nc.tensor.transpose(
                        psc[:, 32 * k:32 * (k + 1)],
                        ch[:, 128 * k:128 * (k + 1)],
                        ist[0:32, :],
                    )
                thT = workp.tile([128, 128], BF16, tag="thT")
                nc.scalar.activation(thT[:], psc[:], AF.Tanh)
                nc.vector.tensor_mul(nxt[:], otT[:], thT[:])

                # off-path: h in normal layout for the output (gpsimd mult)
                thn = workp.tile([96, HID], BF16, tag="thn")
                nc.scalar.activation(thn[64:96, :], ch[:], AF.Tanh)
                nc.gpsimd.tensor_mul(O[32 * tau:32 * (tau + 1), :],
                                     sig[64:96, :], thn[64:96, :])

                if sg == T - 1:
                    cfin = workp.tile([32, HID], F32, tag="cfin")
                    nc.vector.tensor_copy(cfin[:], jc_nxt[32:64, :])
                    nc.gpsimd.dma_start(cout_d.ap()[:, :], cfin[:])

            # ---------------- main pipeline ----------------
            for rep in range(reps):
                nc.vector.memset(hT[0][:], 0.0)
                nc.vector.memset(JC[0][:], 0.0)
                pre_cur, td_cur = alloc_chunk_tiles()
                for r in range(RT):
                    precompute_rowtile(0, r, pre_cur, td_cur)

                for chunk in range(NCHUNK):
                    pre_nxt, td_nxt = (
                        alloc_chunk_tiles() if chunk + 1 < NCHUNK else (None, None)
                    )
                    O = None
                    for s in range(TC):
                        tau = s % 4
                        if tau == 0:
                            O = outp.tile([128, HID], F32, tag="O")
                        step(chunk, s, pre_cur, td_cur, O)
                        if tau == 3:
                            t0 = chunk * TC + (s // 4) * 4
                            nc.gpsimd.dma_start(
                                hseq_d.ap()[:, t0:t0 + 4, :].rearrange(
                                    "b t c -> t b c"),
                                O[:],
                            )
                            if pre_nxt is not None:
                                precompute_rowtile(chunk + 1, s // 4, pre_nxt,
                                                   td_nxt)
                    if pre_nxt is not None:
                        pre_cur, td_cur = pre_nxt, td_nxt

    nc.compile()
    return nc


_CACHE = {}


def _get_nc(reps=1):
    key = ("nc", reps)
    if key not in _CACHE:
        _CACHE[key] = build_bass(reps)
    return _CACHE[key]


def _prep_weights(inp):
    bf = ml_dtypes.bfloat16
    cat = np.concatenate
    wall = cat(
        [inp["Wxi"], inp["Wxf"], inp["Wxo"], inp["Wxc"],
         inp["Wxt1"], inp["Wxt2"], inp["Wxd1"], inp["Wxd2"]], axis=1
    ).astype(bf)
    whf = cat([inp["Whi"], inp["Whf"], inp["Who"], inp["Whc"]], axis=1).astype(bf)
    ones = np.ones((128, 1), np.float32)
    wtb = np.stack(
        [ones * inp["Wt1"], ones * inp["Wt2"], ones * inp["Wd1"],
         ones * inp["Wd2"], ones * inp["Wto"], ones * inp["Wdo"]]
    ).astype(bf)
    biasb = np.stack(
        [ones * inp[k][None, :] for k in
         ["bi", "bf", "bo", "bc", "bt1", "bt2", "bd1", "bd2"]]
    ).astype(bf)
    ist = np.zeros((128, 32), np.float32)
    for tau in range(4):
        ist[32 * tau:32 * (tau + 1)] = np.eye(32)
    ident = np.eye(128, dtype=np.float32)
    return dict(
        wall=np.ascontiguousarray(wall),
        whf=np.ascontiguousarray(whf),
        wtb=np.ascontiguousarray(wtb),
        biasb=np.ascontiguousarray(biasb),
        ist=ist.astype(bf),
        ident=ident.astype(bf),
    )


def run_on_hw(inputs, trace=False, reps=1, **kw):
    nc = _get_nc(reps)
    shared = _prep_weights(inputs)
    x = np.asarray(inputs["input"], np.float32)
    in_maps = []
    for c in range(NCORES):
        m = dict(shared)
        m["x"] = np.ascontiguousarray(x[BS * c:BS * (c + 1)])
        in_maps.append(m)
    res = bass_utils.run_bass_kernel_spmd(
        nc, in_maps, core_ids=list(range(NCORES)), trace=trace, **kw
    )
    return res


def kernel(**inputs):
    res = run_on_hw(inputs)
    hseq = np.concatenate([r["hseq"] for r in res.results], axis=0)
    c_t = np.concatenate([r["c_out"] for r in res.results], axis=0)
    h_t = np.ascontiguousarray(hseq[:, -1, :])
    return hseq, (h_t, c_t)
